# revision 23
# baseline (speedup 1.0000x reference)
"""Relational-GCN layer (gather + degree-normalized segment-mean + MLP head
with BatchNorm) on 8 Trainium2 NeuronCores.

Strategy (edge-parallel, dst-sharded):
  * Nodes sharded contiguously across 8 cores (6272/core = 49 windows of
    128).  Every edge is routed to the core owning its destination, so the
    segment-sum needs no inter-core collective.
  * Per core, edges are grouped by 128-node destination window and split
    into lo/hi source streams (int16 gather-index limit).  Gathered source
    rows (fp16, dma_gather) are scatter-added with an ON-CHIP-generated fp8
    one-hot matmul per 128-edge block accumulating in PSUM; degree
    normalization multiplies by a PE-broadcast 1/deg row afterwards.
  * One-hots are generated on the (otherwise idle) DVE from a tiny int16
    dst-column stream via a broadcast is_equal against an iota row --
    removes the 9.8MB/core one-hot HBM stream that contended with gathers.
  * Gather-index tables load in per-segment chunks so the first gathers
    start ~4us in; gathers round-robin over the 4 SWDGE queues with a
    global counter (queues never reset per segment); xt/cnt bulk loads are
    chunked onto the Activation HWDGE queue so they never head-block the
    Sync queue or starve the gather stream.
  * Per-window block counts are variable (data-derived compile).  To share
    one SPMD program across cores, each core's windows are sorted by edge
    count and matched rank-for-rank (node columns are permuted per core on
    the host; the output is un-permuted afterwards).
  * Relation embeddings enter through a (relation x node) count matrix,
    W1 folds into the GCN weights, BN biases drop analytically.  BN stats
    are reduced on-chip (ACT accum) and AllReduced in two 2KB phases so
    phase 1 overlaps the last segment's compute.
"""

import os
import sys

import numpy as np

sys.path.insert(0, "/opt/trn_rl_repo")

import concourse.bacc as bacc  # noqa: E402
import concourse.mybir as mybir  # noqa: E402
import concourse.tile as tile  # noqa: E402
from concourse.bass_utils import run_bass_kernel_spmd  # noqa: E402

F16 = mybir.dt.float16
F32 = mybir.dt.float32
F8 = mybir.dt.float8e4
I16 = mybir.dt.int16
I8 = mybir.dt.int8

N_ENT = 50000
N_EDGE = 600000
FEAT = 128
HID = 256
OUT = 128
RELS = 101
BN_EPS = 1e-5

CORES = 8
NQ = int(os.environ.get("GNN_NQ", "4"))
W = 128                # nodes per scatter window (one PSUM tile)
NPC = 6272             # nodes per core = 49 * 128 = 14 * 448
NWIN = NPC // W        # 49 windows
NSEG = 7               # segments of 7 windows
WPS = NWIN // NSEG     # windows per segment
CH = 448               # nodes per head chunk
NCH = NPC // CH        # 14 chunks (2 per segment)
LO = 32768             # row split of the gather table (int16 index limit)
HI_ROWS = N_ENT - LO
PADCOL = 300           # dst-column sentinel for padding edges (no one-hot hit)

_compiled = {}
LAST_RESULTS = None    # set by kernel(); test.py reads exec time from here
TRACE = bool(int(os.environ.get("GNN_TRACE", "0")))
NOGATHER = bool(int(os.environ.get("GNN_NOGATHER", "0")))
CUT = os.environ.get("GNN_CUT", "")   # "" | "nohead" | "noscatter"


def _build(k_slot):
    """k_slot: [NWIN, 2] int array of blocks per window-slot per stream."""
    k_lo = k_slot[:, 0]
    k_hi = k_slot[:, 1]
    boff = np.zeros((NWIN + 1, 2), np.int64)
    boff[1:] = np.cumsum(k_slot, axis=0)
    b_lo = int(boff[-1, 0])
    b_hi = int(boff[-1, 1])
    seg_lo = [int(boff[(j + 1) * WPS, 0] - boff[j * WPS, 0]) for j in range(NSEG)]
    seg_hi = [int(boff[(j + 1) * WPS, 1] - boff[j * WPS, 1]) for j in range(NSEG)]
    kseg_lo = max(seg_lo)
    kseg_hi = max(seg_hi)

    nc = bacc.Bacc("TRN2", target_bir_lowering=False, num_devices=CORES,
                   num_swdge_queues=4)

    xlo = nc.dram_tensor("xlo", [LO, FEAT], F16, kind="ExternalInput")
    xhi = nc.dram_tensor("xhi", [HI_ROWS, FEAT], F16, kind="ExternalInput")
    idxlo = nc.dram_tensor("idxlo", [128, b_lo * 8], I16, kind="ExternalInput")
    idxhi = nc.dram_tensor("idxhi", [128, b_hi * 8], I16, kind="ExternalInput")
    dclo = nc.dram_tensor("dclo", [128, b_lo], I8, kind="ExternalInput")
    dchi = nc.dram_tensor("dchi", [128, b_hi], I8, kind="ExternalInput")
    iotain = nc.dram_tensor("iotain", [128, W], I8, kind="ExternalInput")
    reciprow = nc.dram_tensor("reciprow", [1, NPC], F16, kind="ExternalInput")
    onesin = nc.dram_tensor("onesin", [1, 128], F16, kind="ExternalInput")
    xt = nc.dram_tensor("xt", [FEAT, NPC], F16, kind="ExternalInput")
    cnt = nc.dram_tensor("cnt", [RELS, NPC], F16, kind="ExternalInput")
    wmw1 = nc.dram_tensor("wmw1", [FEAT, HID], F16, kind="ExternalInput")
    wsw1 = nc.dram_tensor("wsw1", [FEAT, HID], F16, kind="ExternalInput")
    relw = nc.dram_tensor("relw", [RELS, HID], F16, kind="ExternalInput")
    w2 = nc.dram_tensor("w2", [HID, OUT], F16, kind="ExternalInput")
    smalls = nc.dram_tensor("smalls", [128, 8], F32, kind="ExternalInput")
    outt = nc.dram_tensor("outt", [OUT, NPC], F32, kind="ExternalOutput")

    mul = mybir.AluOpType.mult
    sub = mybir.AluOpType.subtract
    add = mybir.AluOpType.add
    iseq = mybir.AluOpType.is_equal
    AF = mybir.ActivationFunctionType

    with tile.TileContext(nc) as tc:
        with (
            tc.tile_pool(name="sb", bufs=1) as sb,
            tc.tile_pool(name="dbl", bufs=2) as dbl,
            tc.tile_pool(name="gat", bufs=3) as gat,
            tc.tile_pool(name="gat2", bufs=2) as gat2,
            tc.tile_pool(name="ps", bufs=4, space="PSUM") as ps,
            tc.tile_pool(name="psh", bufs=2, space="PSUM") as psh,
            tc.tile_pool(name="dram", bufs=1, space="DRAM") as dram,
        ):
            # ---- idx tables: two chunks per stream (segs 0-1, then 2-6) so
            # the first gathers start early while keeping the DMA
            # instruction count low (the DMA-completion semaphore slots
            # rotate over ALL queues; every extra DMA adds a false
            # full-drain ordering edge against unrelated transfers).
            il_sb = sb.tile([128, b_lo * 8], I16)
            ih_sb = sb.tile([128, b_hi * 8], I16)
            for j in range(NSEG):
                l0, l1 = int(boff[j * WPS, 0]) * 8, int(boff[(j + 1) * WPS, 0]) * 8
                h0, h1 = int(boff[j * WPS, 1]) * 8, int(boff[(j + 1) * WPS, 1]) * 8
                nc.sync.dma_start(il_sb[:, l0:l1], idxlo[:, l0:l1])
                nc.sync.dma_start(ih_sb[:, h0:h1], idxhi[:, h0:h1])

            # ---- small constants + dst-column stream on the ACT queue ----
            dl_sb = sb.tile([128, b_lo], I8)
            dh_sb = sb.tile([128, b_hi], I8)
            io_sb = sb.tile([128, W], I8)
            nc.scalar.dma_start(io_sb[:], iotain[:])
            nc.scalar.dma_start(dl_sb[:], dclo[:])
            nc.scalar.dma_start(dh_sb[:], dchi[:])
            sm = sb.tile([128, 8], F32)
            nc.scalar.dma_start(sm[:], smalls[:])
            rr_sb = sb.tile([1, NPC], F16)
            nc.scalar.dma_start(rr_sb[:], reciprow[:])
            ones1 = sb.tile([1, 128], F16)
            nc.scalar.dma_start(ones1[:], onesin[:])
            wm_sb = sb.tile([FEAT, HID], F16)
            nc.scalar.dma_start(wm_sb[:], wmw1[:])
            ws_sb = sb.tile([FEAT, HID], F16)
            nc.scalar.dma_start(ws_sb[:], wsw1[:])
            rw_sb = sb.tile([RELS, HID], F16)
            nc.scalar.dma_start(rw_sb[:], relw[:])
            w2_sb = [sb.tile([128, OUT], F16, tag=f"w2_{h}", name=f"w2sb{h}")
                     for h in range(2)]
            for h in range(2):
                nc.scalar.dma_start(w2_sb[h][:], w2[h * 128:(h + 1) * 128, :])
            # xt/cnt quarters are issued inside the segment loop (segs 0-3)
            # so they trickle in behind the gather stream.
            xt_sb = sb.tile([FEAT, NPC], F16)
            cn_sb = sb.tile([RELS, NPC], F16)

            # hoisted num_idxs registers: one MOVE per distinct row count,
            # so per-gather MOVEs don't occupy the Pool engine's shallow
            # instruction lookahead window (the next wave's desc-gen must
            # enter it while the current wave's leader blocks on its drain)
            nreg = {}

            aggn = sb.tile([128, NPC], F16)
            yt = [[None] * NCH, [None] * NCH]
            sump = [sb.tile([128, NCH], F32, tag=f"sump{h}", name=f"sump{h}")
                    for h in range(2)]
            sqp = [sb.tile([128, NCH], F32, tag=f"sqp{h}", name=f"sqp{h}")
                   for h in range(2)]

            st = sb.tile([128, 8], F32)
            bin1 = dram.tile([128, 4], F32, tag="bin1", name="bin1")
            bout1 = dram.tile([128, 4], F32, tag="bout1", name="bout1")
            bin2 = dram.tile([128, 4], F32, tag="bin2", name="bin2")
            bout2 = dram.tile([128, 4], F32, tag="bout2", name="bout2")

            # ---- 1/deg broadcast along partitions (K=1 matmul trick) ----
            rbful = sb.tile([128, NPC], F16)
            for s in range(NWIN):
                rp = ps.tile([128, W], F32, tag="ps")
                nc.tensor.matmul(rp[:], lhsT=ones1[:],
                                 rhs=rr_sb[:, s * W:(s + 1) * W],
                                 start=True, stop=True)
                nc.vector.tensor_scalar(rbful[:, s * W:(s + 1) * W],
                                        rp[:], 1.0, None, mul)

            # ---- per-segment: gather (4 queues) -> scatter -> head pass A ----
            gq = 0
            for j in range(NSEG):
                lb0 = int(boff[j * WPS, 0])
                hb0 = int(boff[j * WPS, 1])
                nlo = seg_lo[j]
                nhi = seg_hi[j]
                qlo = (kseg_lo + 3) // 4
                qhi = (kseg_hi + 3) // 4
                gm_lo = [gat.tile([128, qlo, FEAT], F16, tag=f"glo{q}",
                                  name=f"glo{q}_{j}") for q in range(4)]
                gm_hi = [gat.tile([128, qhi, FEAT], F16, tag=f"ghi{q}",
                                  name=f"ghi{q}_{j}") for q in range(4)]
                # 8 sub-gathers per segment (4 per stream) into SEPARATE
                # quarter tiles, global round-robin over the 4 SWDGE queues:
                # rings stay fed across segment boundaries, and a window's
                # matmuls depend only on the quarter tiles holding its blocks.
                lob = [nlo * i // 4 for i in range(5)]
                hib = [nhi * i // 4 for i in range(5)]
                lmap = []
                hmap = []
                parts = []
                for i4 in range(4):
                    parts.append((gm_lo[i4], xlo, il_sb, lb0 + lob[i4],
                                  lob[i4 + 1] - lob[i4], gq % NQ))
                    lmap += [(gm_lo[i4], b) for b in range(lob[i4 + 1] - lob[i4])]
                    gq += 1
                    parts.append((gm_hi[i4], xhi, ih_sb, hb0 + hib[i4],
                                  hib[i4 + 1] - hib[i4], gq % NQ))
                    hmap += [(gm_hi[i4], b) for b in range(hib[i4 + 1] - hib[i4])]
                    gq += 1
                for gm, xtab, isb, base, nb, q in parts:
                    if nb <= 0:
                        continue
                    if NOGATHER:
                        nc.vector.memset(gm[:, :nb, :], 0.25)
                    else:
                        if nb * 128 not in nreg:
                            nreg[nb * 128] = nc.gpsimd.to_reg(nb * 128)
                        nc.gpsimd.dma_gather(
                            gm[:, :nb, :], xtab[:],
                            isb[:, base * 8:(base + nb) * 8],
                            nb * 128, nreg[nb * 128], FEAT,
                            single_packet=False, queue_num=q)
                if j < 4:
                    qs = slice(j * (NPC // 4), (j + 1) * (NPC // 4))
                    nc.scalar.dma_start(xt_sb[:, qs], xt[:, qs])
                    nc.scalar.dma_start(cn_sb[:, qs], cnt[:, qs])

                # ---- on-chip one-hot generation (DVE broadcast is_equal) ----
                oh_lo = gat2.tile([128, kseg_lo, W], F8, tag="olo")
                oh_hi = gat2.tile([128, kseg_hi, W], F8, tag="ohi")
                if CUT != "noscatter":
                    io_b = io_sb[:].unsqueeze(1)
                    nc.vector.tensor_tensor(
                        out=oh_lo[:, :nlo, :],
                        in0=io_b.broadcast_to((128, nlo, W)),
                        in1=dl_sb[:, lb0:lb0 + nlo].unsqueeze(2)
                            .broadcast_to((128, nlo, W)),
                        op=iseq)
                    nc.vector.tensor_tensor(
                        out=oh_hi[:, :nhi, :],
                        in0=io_b.broadcast_to((128, nhi, W)),
                        in1=dh_sb[:, hb0:hb0 + nhi].unsqueeze(2)
                            .broadcast_to((128, nhi, W)),
                        op=iseq)

                if CUT == "noscatter":
                    nc.vector.memset(aggn[:, j * WPS * W:(j + 1) * WPS * W], 0.125)
                for wi in range(WPS if CUT != "noscatter" else 0):
                    s = j * WPS + wi
                    lo_off = int(boff[s, 0]) - lb0
                    hi_off = int(boff[s, 1]) - hb0
                    acc = ps.tile([128, W], F32, tag="ps")
                    for b in range(int(k_lo[s])):
                        gt, gb = lmap[lo_off + b]
                        nc.tensor.matmul(acc[:],
                                         lhsT=gt[:, gb, :],
                                         rhs=oh_lo[:, lo_off + b, :],
                                         start=(b == 0), stop=False)
                    for b in range(int(k_hi[s])):
                        gt, gb = hmap[hi_off + b]
                        nc.tensor.matmul(acc[:],
                                         lhsT=gt[:, gb, :],
                                         rhs=oh_hi[:, hi_off + b, :],
                                         start=False, stop=(b == int(k_hi[s]) - 1))
                    nc.vector.tensor_tensor(
                        out=aggn[:, s * W:(s + 1) * W], in0=acc[:],
                        in1=rbful[:, s * W:(s + 1) * W], op=mul)

                if CUT == "nohead":
                    continue
                # head pass A for the 2 chunks of this segment
                for c in (2 * j, 2 * j + 1):
                    cs = slice(c * CH, (c + 1) * CH)
                    for h in range(2):
                        hs = slice(h * 128, (h + 1) * 128)
                        yp = psh.tile([128, CH], F32, tag="psA")
                        nc.tensor.matmul(yp[:], lhsT=wm_sb[:, hs],
                                         rhs=aggn[:, cs], start=True, stop=False)
                        nc.tensor.matmul(yp[:], lhsT=ws_sb[:, hs],
                                         rhs=xt_sb[:, cs], start=False, stop=False)
                        nc.tensor.matmul(yp[:], lhsT=rw_sb[:, hs],
                                         rhs=cn_sb[:, cs], start=False, stop=True)
                        ytile = sb.tile([128, CH], F16, tag=f"y{h}_{c}",
                                        name=f"y{h}_{c}")
                        nc.scalar.activation(ytile[:], yp[:], AF.Copy,
                                             accum_out=sump[h][:, c:c + 1])
                        yt[h][c] = ytile
                        sq = dbl.tile([128, CH], F16, tag="sqt")
                        nc.scalar.activation(sq[:], ytile[:], AF.Square,
                                             accum_out=sqp[h][:, c:c + 1])

            if CUT == "nohead":
                for c in range(NCH):
                    ost = dbl.tile([OUT, CH], F32, tag="ost")
                    nc.scalar.copy(ost[:], aggn[:, c * CH:(c + 1) * CH])
                    nc.sync.dma_start(outt[:, c * CH:(c + 1) * CH], ost[:])
            else:
                # ---- BN stats: single AllReduce over all 14 chunks (the
                # phase-1/phase-2 split bought nothing: the AR only runs once
                # every core's gather stream ends, so both phases fired
                # back-to-back in the tail anyway) ----
                for h in range(2):
                    nc.vector.tensor_reduce(st[:, h:h + 1], sump[h][:, :],
                                            axis=mybir.AxisListType.X, op=add)
                    nc.vector.tensor_reduce(st[:, 2 + h:3 + h], sqp[h][:, :],
                                            axis=mybir.AxisListType.X, op=add)
                nc.sync.dma_start(bin1[:], st[:, 0:4])
                nc.gpsimd.collective_compute(
                    "AllReduce", add,
                    replica_groups=[list(range(CORES))],
                    ins=[bin1.opt()], outs=[bout1.opt()])
                ar = sb.tile([128, 4], F32)
                nc.sync.dma_start(ar[:], bout1[:])

                # ---- scale/shift: s = gamma/sqrt(var+eps), t = beta - mean*s
                prm = sb.tile([128, 10], F32)
                inv_n = 1.0 / float(N_ENT)
                nc.vector.tensor_scalar(prm[:, 0:2], ar[:, 0:2], inv_n, None, mul)
                nc.vector.tensor_scalar(prm[:, 2:4], ar[:, 2:4], inv_n, None, mul)
                nc.vector.tensor_tensor(prm[:, 4:6], prm[:, 0:2], prm[:, 0:2], mul)
                nc.vector.tensor_tensor(prm[:, 6:8], prm[:, 2:4], prm[:, 4:6], sub)
                nc.vector.tensor_scalar(prm[:, 6:8], prm[:, 6:8], BN_EPS, None, add)
                sd = sb.tile([128, 2], F32)
                nc.scalar.sqrt(sd[:], prm[:, 6:8])
                rsd = sb.tile([128, 2], F32)
                nc.vector.reciprocal(rsd[:], sd[:])
                sc = sb.tile([128, 2], F32)
                nc.vector.tensor_tensor(sc[:], rsd[:], sm[:, 0:2], mul)
                tmp = sb.tile([128, 2], F32)
                nc.vector.tensor_tensor(tmp[:], prm[:, 0:2], sc[:], mul)
                tf = sb.tile([128, 2], F32)
                nc.vector.tensor_tensor(tf[:], sm[:, 2:4], tmp[:], sub)

                # ---- head pass B: relu(s*y'+t) @ W2 + b2 ----
                # half 0 on the ACT engine, half 1 on the DVE: the two relus
                # per chunk run in parallel instead of serializing on ACT
                mx = mybir.AluOpType.max
                for c in range(NCH):
                    op = psh.tile([128, CH], F32, tag="psB")
                    for h in range(2):
                        yr = dbl.tile([128, CH], F16, tag=f"yr{h}")
                        if h == 0:
                            nc.scalar.activation(yr[:], yt[h][c][:], AF.Relu,
                                                 bias=tf[:, h:h + 1],
                                                 scale=sc[:, h:h + 1])
                        else:
                            nc.vector.tensor_scalar(yr[:], yt[h][c][:],
                                                    sc[:, h:h + 1],
                                                    tf[:, h:h + 1], mul, add)
                            nc.vector.tensor_scalar(yr[:], yr[:], 0.0, None, mx)
                        nc.tensor.matmul(op[:], lhsT=w2_sb[h][:], rhs=yr[:],
                                         start=(h == 0), stop=(h == 1))
                    ost = dbl.tile([OUT, CH], F32, tag="ost")
                    nc.vector.tensor_scalar(ost[:], op[:], sm[:, 4:5], None, add)
                    nc.sync.dma_start(outt[:, c * CH:(c + 1) * CH], ost[:])

    nc.compile()
    return nc


def _prep(edge_index, edge_type):
    src = edge_index[0].astype(np.int64)
    dst = edge_index[1].astype(np.int64)
    et = edge_type.astype(np.int64)
    deg = np.bincount(dst, minlength=N_ENT)
    recip32 = (1.0 / np.maximum(deg, 1.0)).astype(np.float32)

    cntm = np.bincount(dst * RELS + et, minlength=N_ENT * RELS)
    cntm = cntm.reshape(N_ENT, RELS).astype(np.float32)
    cn_full = (cntm * recip32[:, None]).T.astype(np.float16)   # [101, N]

    core = dst // NPC
    win = (dst % NPC) // W
    stream = (src >= LO).astype(np.int64)

    counts = np.bincount(((core * NWIN + win) * 2 + stream),
                         minlength=CORES * NWIN * 2).reshape(CORES, NWIN, 2)
    tot = counts.sum(2)                                        # [8, 49]
    # rank windows per core by total count (desc); rank r -> slot (r%7)*7+r//7
    permw = np.argsort(-tot, axis=1, kind="stable")            # [8, rank] -> win
    rank_of_slot = np.arange(NWIN).reshape(WPS, NSEG).T.reshape(-1)
    win_at_slot = permw[:, rank_of_slot]                       # [8, slot] -> win
    slot_of_win = np.empty_like(win_at_slot)
    for c in range(CORES):
        slot_of_win[c, win_at_slot[c]] = np.arange(NWIN)

    cnt_slot = counts[np.arange(CORES)[:, None], win_at_slot, :]  # [8, 49, 2]
    k_slot = np.maximum(((cnt_slot + 127) // 128).max(0), 1).astype(np.int64)  # [49,2]
    boff = np.zeros((NWIN + 1, 2), np.int64)
    boff[1:] = np.cumsum(k_slot, axis=0)
    b_lo, b_hi = int(boff[-1, 0]), int(boff[-1, 1])

    # per-edge slot assignment
    slot = slot_of_win[core, win]
    key = (core * NWIN + slot) * 2 + stream
    order = np.argsort(key, kind="stable")
    skey = key[order]
    kcounts = np.bincount(key, minlength=CORES * NWIN * 2)
    starts = np.zeros(CORES * NWIN * 2, np.int64)
    np.cumsum(kcounts[:-1], out=starts[1:])
    rank = np.arange(N_EDGE) - starts[skey]
    g_slot = (skey // 2) % NWIN
    g_core = skey // (2 * NWIN)
    g_str = skey % 2
    gpos = (boff[g_slot, g_str] + rank // 128) * 128 + rank % 128

    ssrc = src[order]
    sdst = dst[order]
    sdcol = (sdst % NPC) % W

    n_lo = b_lo * 128
    n_hi = b_hi * 128
    idx_lo = np.zeros((CORES, n_lo), np.int16)
    idx_hi = np.zeros((CORES, n_hi), np.int16)
    dc_lo = np.full((CORES, n_lo), -1, np.int8)
    dc_hi = np.full((CORES, n_hi), -1, np.int8)
    lo = g_str == 0
    hi = ~lo
    idx_lo[g_core[lo], gpos[lo]] = ssrc[lo].astype(np.int16)
    dc_lo[g_core[lo], gpos[lo]] = sdcol[lo].astype(np.int8)
    idx_hi[g_core[hi], gpos[hi]] = (ssrc[hi] - LO).astype(np.int16)
    dc_hi[g_core[hi], gpos[hi]] = sdcol[hi].astype(np.int8)

    def wrap_idx(a):
        # element i -> [i % 16, i // 16], replicated over the 8 Q7 cores
        w = a.reshape(-1, 16).T
        return np.tile(w, (8, 1)).copy()

    def dc_dev(a, b):
        # slot i = b*128+p -> [p, b]
        return np.ascontiguousarray(a.reshape(b, 128).T)

    per_core = []
    for c in range(CORES):
        per_core.append({
            "idxlo": wrap_idx(idx_lo[c]),
            "idxhi": wrap_idx(idx_hi[c]),
            "dclo": dc_dev(dc_lo[c], b_lo),
            "dchi": dc_dev(dc_hi[c], b_hi),
        })
    pad = (n_lo + n_hi) * CORES / N_EDGE - 1.0
    return per_core, cn_full, recip32, k_slot, win_at_slot, pad


def kernel(edge_index, edge_type, initial_features, relation_embeddings,
           W_msg, b_msg, W_self, W1, b1, gamma, beta, W2, b2):
    global LAST_RESULTS
    edge_index = np.asarray(edge_index)
    edge_type = np.asarray(edge_type)
    x = np.asarray(initial_features, dtype=np.float32)

    per_core, cn_full, recip32, k_slot, win_at_slot, pad = _prep(edge_index, edge_type)

    x16 = x.astype(np.float16)
    xlo_t = np.ascontiguousarray(x16[:LO])
    xhi_t = np.ascontiguousarray(x16[LO:])

    Wm = np.asarray(W_msg, np.float64)
    Ws = np.asarray(W_self, np.float64)
    W1_ = np.asarray(W1, np.float64)
    rel = np.asarray(relation_embeddings, np.float64)
    wmw1 = np.ascontiguousarray((Wm @ W1_).astype(np.float16))
    wsw1 = np.ascontiguousarray((Ws @ W1_).astype(np.float16))
    relw = np.ascontiguousarray((rel @ Wm @ W1_).astype(np.float16))
    w2_16 = np.asarray(W2, np.float16)

    smalls = np.zeros((128, 8), np.float32)
    g = np.asarray(gamma, np.float32)
    b = np.asarray(beta, np.float32)
    smalls[:, 0] = g[:128]
    smalls[:, 1] = g[128:]
    smalls[:, 2] = b[:128]
    smalls[:, 3] = b[128:]
    smalls[:, 4] = np.asarray(b2, np.float32)

    iota_np = np.tile(np.arange(W, dtype=np.int8), (128, 1)).copy()

    # per-core permuted node order: slot s holds window win_at_slot[c, s]
    x16p = np.zeros((CORES, NPC, FEAT), np.float16)
    cn_p = np.zeros((CORES, RELS, NPC), np.float16)
    perms = []
    rr_p = []
    ones_np = np.ones((1, 128), np.float16)
    for c in range(CORES):
        pn = (win_at_slot[c][:, None] * W + np.arange(W)[None, :]).reshape(-1)
        gn = c * NPC + pn
        valid = gn < N_ENT
        perms.append((pn, gn, valid))
        x16p[c][valid] = x16[gn[valid]]
        cn_p[c][:, valid] = cn_full[:, gn[valid]]
        rr = np.ones((1, NPC), np.float16)
        rr[0, valid] = recip32[gn[valid]].astype(np.float16)
        rr_p.append(rr)

    in_maps = []
    for c in range(CORES):
        in_maps.append({
            "xlo": xlo_t, "xhi": xhi_t,
            "xt": np.ascontiguousarray(x16p[c].T), "cnt": cn_p[c],
            "reciprow": rr_p[c], "onesin": ones_np, "iotain": iota_np,
            "wmw1": wmw1, "wsw1": wsw1, "relw": relw, "w2": w2_16,
            "smalls": smalls,
            **per_core[c],
        })

    ckey = k_slot.tobytes()
    if ckey not in _compiled:
        _compiled[ckey] = _build(k_slot)
    nc = _compiled[ckey]

    res = run_bass_kernel_spmd(nc, in_maps, list(range(CORES)), trace=TRACE)
    LAST_RESULTS = res

    out = np.zeros((N_ENT, OUT), np.float32)
    for c in range(CORES):
        pn, gn, valid = perms[c]
        out[gn[valid]] = res.results[c]["outt"].T[valid]
    return out
